# revision 14
# baseline (speedup 1.0000x reference)
"""Multihead causal attention on 8 TRN2 NeuronCores.

Sharding: core = (batch b, head-group hg): b = core//2, hg = core%2.
Each core gets x[b] (full sequence, [2048, 1024]) plus the weight rows for
its 8 heads (W[hg*512:(hg+1)*512, :]), computes Q/K/V projections and
causal attention for those (batch, head) pairs, and writes Y transposed
as [8, 64, 2048] (head, dh, seq) in bf16; the host transposes back and
upcasts on gather.

On-device dataflow (per core):
  - x.T supplied pre-transposed by the host (bf16), weights pre-transposed
    likewise; all matmuls bf16.
  - Projections interleaved with attention per head-pair g: K/Q chunks for
    (g, q-chunk qt) are projected just before attention consumes them, so
    the Scalar engine's exp stream starts ~4us into the kernel instead of
    after all projections.
  - Scores in transposed layout scoresT[k, q] = K @ Q.T per head, two heads
    per 128-row group (K=64 each). One [128, 1024] PSUM tile per k-tile
    holds BOTH heads' scores (cols 0:512 head A, 512:1024 head B), so the
    psS pool's two buffers give one full k-tile of score/exp slack and the
    PE never stalls on the exp of the previous k-tile.
  - Causal trimming at 128-column granularity: diagonal k-tiles only
    compute/exp/stream q >= k, so no zero-prefix memsets are needed (PV
    streams are trimmed identically and never read the unwritten prefix).
  - Softmax without a max pass; exp on ScalarE (PSUM -> bf16 SBUF), one
    instruction per off-diagonal k-tile covering both heads; diagonal
    128x128 blocks masked with a triangular 0/1 multiply on DVE.
  - PV matmul in bf16 with a ones-column appended to V: out [65, 512]
    rows 0..63 = unnormalized Y.T, row 64 = softmax denominator.
  - Normalize: denominator copy + fast reciprocal on DVE (reciprocal must
    not read PSUM directly on hardware), partition-broadcast on GpSimd,
    one DVE multiply (fp32 -> bf16), DMA out. The final step is normalized
    in four column chunks to keep the tail off the single-queue DMA rate.
"""
import numpy as np
import ml_dtypes

import concourse.bass as bass
import concourse.tile as tile
from concourse import bacc, mybir
from concourse.bass_utils import run_bass_kernel_spmd

F32 = mybir.dt.float32
BF16 = mybir.dt.bfloat16
EXP = mybir.ActivationFunctionType.Exp

B, S, D, H, DH = 4, 2048, 1024, 16, 64
N_CORES = 8
H_LOC = 8          # heads per core
D_LOC = H_LOC * DH  # 512: projection output dim per core
N_CT = D // 128     # 8 contraction tiles
N_ST = S // 128     # 16 sequence tiles of 128
N_QT = S // 512     # 4 q-chunks of 512
SCALE = 1.0 / np.sqrt(DH)

_NC_CACHE = {}


def build_nc():
    nc = bacc.Bacc("TRN2", target_bir_lowering=False, debug=False,
                   num_devices=N_CORES)
    xtd = nc.dram_tensor("xtd", [D, S], BF16, kind="ExternalInput").ap()
    wqt = nc.dram_tensor("wqt", [D, D_LOC], BF16, kind="ExternalInput").ap()
    wkt = nc.dram_tensor("wkt", [D, D_LOC], BF16, kind="ExternalInput").ap()
    wvt = nc.dram_tensor("wvt", [D, D_LOC], BF16, kind="ExternalInput").ap()
    out = nc.dram_tensor("out", [H_LOC, DH, S], BF16,
                         kind="ExternalOutput").ap()

    # tri[kk, qq] = 1 iff qq >= kk (valid: query position >= key position)
    tri_np = (np.arange(128)[None, :] >= np.arange(128)[:, None])
    tri_dram = nc.inline_tensor(tri_np.astype(ml_dtypes.bfloat16), name="tri")

    with tile.TileContext(nc) as tc:
        with tc.tile_pool(name="consts", bufs=1) as consts, \
             tc.tile_pool(name="pers", bufs=1) as pers, \
             tc.tile_pool(name="xw", bufs=1) as xw, \
             tc.tile_pool(name="epool", bufs=8) as epool, \
             tc.tile_pool(name="norm", bufs=4) as norm, \
             tc.tile_pool(name="psP", bufs=2, space="PSUM") as psP, \
             tc.tile_pool(name="psS", bufs=2, space="PSUM") as psS, \
             tc.tile_pool(name="psY", bufs=1, space="PSUM") as psY:
            tri = consts.tile([128, 128], BF16)
            nc.sync.dma_start(tri[:], tri_dram.ap())

            # input tiles (persistent): x.T and the three weights
            xT = [xw.tile([128, S], BF16, tag=f"xT{i}", name=f"xT{i}")
                  for i in range(N_CT)]
            WK = [xw.tile([128, D_LOC], BF16, tag=f"WK{i}", name=f"WK{i}")
                  for i in range(N_CT)]
            WQ = [xw.tile([128, D_LOC], BF16, tag=f"WQ{i}", name=f"WQ{i}")
                  for i in range(N_CT)]
            WV = [xw.tile([128, D_LOC], BF16, tag=f"WV{i}", name=f"WV{i}")
                  for i in range(N_CT)]
            # K first (needed first), interleaved with x
            for ct in range(N_CT):
                nc.sync.dma_start(xT[ct][:], xtd[ct * 128:(ct + 1) * 128, :])
                nc.sync.dma_start(WK[ct][:], wkt[ct * 128:(ct + 1) * 128, :])
            for ct in range(N_CT):
                nc.sync.dma_start(WQ[ct][:], wqt[ct * 128:(ct + 1) * 128, :])
            for ct in range(N_CT):
                nc.sync.dma_start(WV[ct][:], wvt[ct * 128:(ct + 1) * 128, :])

            # persistent per-core tensors
            QT = [pers.tile([128, S], BF16, tag=f"QT{i}", name=f"QT{i}")
                  for i in range(4)]
            KT = [pers.tile([128, S], BF16, tag=f"KT{i}", name=f"KT{i}")
                  for i in range(4)]
            VP = [pers.tile([128, H_LOC, DH + 1], BF16, tag=f"VP{i}",
                            name=f"VP{i}") for i in range(N_ST)]

            def proj_qk(dst, W, g, qc):
                # dst[g][:, qc*512:(qc+1)*512] = W[:, g-block].T @ x.T chunk
                pp = psP.tile([128, 512], F32, tag="pp", name="pp")
                for ct in range(N_CT):
                    nc.tensor.matmul(
                        pp[:],
                        W[ct][:, g * 128:(g + 1) * 128],
                        xT[ct][:, qc * 512:(qc + 1) * 512],
                        start=(ct == 0), stop=(ct == N_CT - 1))
                nc.vector.tensor_copy(
                    dst[g][:, qc * 512:(qc + 1) * 512], pp[:])

            def proj_v(st):
                pp = psP.tile([128, 512], F32, tag="pp", name="pp")
                for ct in range(N_CT):
                    nc.tensor.matmul(
                        pp[:],
                        xT[ct][:, st * 128:(st + 1) * 128],
                        WV[ct][:],
                        start=(ct == 0), stop=(ct == N_CT - 1))
                nc.vector.tensor_copy(
                    VP[st][:, :, 0:DH],
                    pp[:].rearrange("p (h d) -> p h d", h=H_LOC))
                nc.vector.memset(VP[st][:, :, DH:DH + 1], 1.0)

            def next_step(g, qt):
                if qt < N_QT - 1:
                    return (g, qt + 1)
                if g < 3:
                    return (g + 1, 0)
                return None

            # pipeline fill: first attention step's K/Q chunks
            proj_qk(KT, WK, 0, 0)
            proj_qk(QT, WQ, 0, 0)

            for g in range(4):          # head pair: local heads 2g, 2g+1
                for qt in range(N_QT):  # q-chunk of 512
                    n_kt = 4 * (qt + 1)
                    q0 = qt * 512
                    last = (g == 3 and qt == N_QT - 1)
                    yy = [psY.tile([DH + 1, 512], F32, tag=f"y{hh}",
                                   name=f"y{hh}") for hh in range(2)]

                    def norm_emit(hh, c0, c1):
                        w = c1 - c0
                        den = norm.tile([1, w], F32, tag="den", name="den")
                        nc.vector.tensor_copy(den[:], yy[hh][DH:DH + 1, c0:c1])
                        rd = norm.tile([1, w], F32, tag="rd", name="rd")
                        nc.vector.reciprocal_approx_fast(rd[:], den[:])
                        rdb = norm.tile([DH, w], F32, tag="rdb", name="rdb")
                        nc.gpsimd.partition_broadcast(rdb[:], rd[:])
                        yn = norm.tile([DH, w], BF16, tag="yn", name="yn")
                        nc.vector.tensor_mul(yn[:], yy[hh][0:DH, c0:c1],
                                             rdb[:])
                        nc.sync.dma_start(
                            out[2 * g + hh, :, q0 + c0:q0 + c1], yn[:])

                    for kt in range(n_kt):
                        off = max(0, (kt - 4 * qt) * 128)
                        diag = kt >= 4 * qt
                        # one PSUM tile per k-tile: cols 0:512 head A,
                        # 512:1024 head B
                        ps2 = psS.tile([128, 1024], F32, tag="s", name="s")
                        for hh in range(2):
                            rows = slice(hh * 64, hh * 64 + 64)
                            nc.tensor.matmul(
                                ps2[:, hh * 512 + off:(hh + 1) * 512],
                                KT[g][rows, kt * 128:(kt + 1) * 128],
                                QT[g][rows, q0 + off:q0 + 512],
                                start=True, stop=True)
                        if kt == 0:
                            # PE work that overlaps the exp latency of the
                            # scores just issued: V projections (g==0 only)
                            # and the next attention step's K/Q chunks.
                            if g == 0:
                                for st in range(4 * qt, 4 * qt + 2):
                                    proj_v(st)
                            nxt = next_step(g, qt)
                            if nxt is not None:
                                proj_qk(KT, WK, nxt[0], nxt[1])
                        if kt == 1:
                            if g == 0:
                                for st in range(4 * qt + 2, 4 * qt + 4):
                                    proj_v(st)
                            nxt = next_step(g, qt)
                            if nxt is not None:
                                proj_qk(QT, WQ, nxt[0], nxt[1])
                        ee = epool.tile([128, 1024], BF16, tag="e", name="e")
                        if not diag:
                            nc.scalar.activation(ee[:], ps2[:], EXP,
                                                 scale=SCALE)
                        else:
                            for hh in range(2):
                                cb = hh * 512
                                nc.scalar.activation(
                                    ee[:, cb + off:cb + 512],
                                    ps2[:, cb + off:cb + 512],
                                    EXP, scale=SCALE)
                                nc.vector.tensor_mul(
                                    ee[:, cb + off:cb + off + 128],
                                    ee[:, cb + off:cb + off + 128],
                                    tri[:])
                        for hh in range(2):
                            nc.tensor.matmul(
                                yy[hh][:, off:512],
                                VP[kt][:, 2 * g + hh, :],
                                ee[:, hh * 512 + off:(hh + 1) * 512],
                                start=(kt == 0), stop=(kt == n_kt - 1),
                                skip_group_check=True)
                        if last and diag:
                            # final step: normalize each 128-col block as
                            # soon as its last PV contribution lands, so
                            # only one small chain remains after the last
                            # matmul
                            j = kt - 4 * qt
                            for hh in range(2):
                                norm_emit(hh, j * 128, (j + 1) * 128)

                    if not last:
                        for hh in range(2):
                            norm_emit(hh, 0, 512)
    nc.compile()
    return nc


def get_nc():
    if "nc" not in _NC_CACHE:
        _NC_CACHE["nc"] = build_nc()
    return _NC_CACHE["nc"]


def make_in_maps(x, W_q, W_k, W_v):
    in_maps = []
    for core in range(N_CORES):
        b, hg = core // 2, core % 2
        rows = slice(hg * D_LOC, (hg + 1) * D_LOC)
        bf = ml_dtypes.bfloat16
        in_maps.append({
            "xtd": np.ascontiguousarray(np.asarray(x[b], dtype=np.float32).T.astype(bf)),
            "wqt": np.ascontiguousarray(np.asarray(W_q[rows], dtype=np.float32).T.astype(bf)),
            "wkt": np.ascontiguousarray(np.asarray(W_k[rows], dtype=np.float32).T.astype(bf)),
            "wvt": np.ascontiguousarray(np.asarray(W_v[rows], dtype=np.float32).T.astype(bf)),
        })
    return in_maps


def assemble(results):
    Y = np.empty((B, H, S, DH), dtype=np.float32)
    for core in range(N_CORES):
        b, hg = core // 2, core % 2
        yc = np.asarray(results[core]["out"], dtype=np.float32)  # [H_LOC, DH, S]
        Y[b, hg * H_LOC:(hg + 1) * H_LOC] = yc.transpose(0, 2, 1)
    return Y


def kernel(x, W_q, W_k, W_v):
    nc = get_nc()
    in_maps = make_in_maps(x, W_q, W_k, W_v)
    res = run_bass_kernel_spmd(nc, in_maps, list(range(N_CORES)))
    return assemble(res.results)


# revision 15
# speedup vs baseline: 1.0261x; 1.0261x over previous
"""Multihead causal attention on 8 TRN2 NeuronCores.

Sharding: core = (batch b, head-group hg): b = core//2, hg = core%2.
Each core gets x[b] (full sequence, [2048, 1024]) plus the weight rows for
its 8 heads (W[hg*512:(hg+1)*512, :]), computes Q/K/V projections and
causal attention for those (batch, head) pairs, and writes Y transposed
as [8, 64, 2048] (head, dh, seq) in bf16; the host transposes back and
upcasts on gather.

On-device dataflow (per core):
  - x.T supplied pre-transposed by the host (bf16), weights pre-transposed
    likewise; all matmuls bf16.
  - Projections interleaved with attention per head-pair g: K/Q chunks for
    (g, q-chunk qt) are projected just before attention consumes them, so
    the Scalar engine's exp stream starts ~4us into the kernel instead of
    after all projections.
  - Scores in transposed layout scoresT[k, q] = K @ Q.T per head, two heads
    per 128-row group (K=64 each). One [128, 1024] PSUM tile per k-tile
    holds BOTH heads' scores (cols 0:512 head A, 512:1024 head B), so the
    psS pool's two buffers give one full k-tile of score/exp slack and the
    PE never stalls on the exp of the previous k-tile.
  - Causal trimming at 128-column granularity: diagonal k-tiles only
    compute/exp/stream q >= k, so no zero-prefix memsets are needed (PV
    streams are trimmed identically and never read the unwritten prefix).
  - Softmax without a max pass; exp on ScalarE (PSUM -> bf16 SBUF), one
    instruction per off-diagonal k-tile covering both heads; diagonal
    128x128 blocks masked with a triangular 0/1 multiply on DVE.
  - PV matmul in bf16 with a ones-column appended to V: out [65, 512]
    rows 0..63 = unnormalized Y.T, row 64 = softmax denominator.
  - Normalize: denominator copy + fast reciprocal on DVE (reciprocal must
    not read PSUM directly on hardware), partition-broadcast on GpSimd,
    one DVE multiply (fp32 -> bf16), DMA out. The final step is normalized
    in four column chunks to keep the tail off the single-queue DMA rate.
"""
import numpy as np
import ml_dtypes

import concourse.bass as bass
import concourse.tile as tile
from concourse import bacc, mybir
from concourse.bass_utils import run_bass_kernel_spmd

F32 = mybir.dt.float32
BF16 = mybir.dt.bfloat16
EXP = mybir.ActivationFunctionType.Exp

B, S, D, H, DH = 4, 2048, 1024, 16, 64
N_CORES = 8
H_LOC = 8          # heads per core
D_LOC = H_LOC * DH  # 512: projection output dim per core
N_CT = D // 128     # 8 contraction tiles
N_ST = S // 128     # 16 sequence tiles of 128
N_QT = S // 512     # 4 q-chunks of 512
SCALE = 1.0 / np.sqrt(DH)

_NC_CACHE = {}


def build_nc():
    nc = bacc.Bacc("TRN2", target_bir_lowering=False, debug=False,
                   num_devices=N_CORES)
    xtd = nc.dram_tensor("xtd", [D, S], BF16, kind="ExternalInput").ap()
    wqt = nc.dram_tensor("wqt", [D, D_LOC], BF16, kind="ExternalInput").ap()
    wkt = nc.dram_tensor("wkt", [D, D_LOC], BF16, kind="ExternalInput").ap()
    wvt = nc.dram_tensor("wvt", [D, D_LOC], BF16, kind="ExternalInput").ap()
    out = nc.dram_tensor("out", [H_LOC, DH, S], BF16,
                         kind="ExternalOutput").ap()

    # tri[kk, qq] = 1 iff qq >= kk (valid: query position >= key position)
    tri_np = (np.arange(128)[None, :] >= np.arange(128)[:, None])
    tri_dram = nc.inline_tensor(tri_np.astype(ml_dtypes.bfloat16), name="tri")

    with tile.TileContext(nc) as tc:
        with tc.tile_pool(name="consts", bufs=1) as consts, \
             tc.tile_pool(name="pers", bufs=1) as pers, \
             tc.tile_pool(name="xw", bufs=1) as xw, \
             tc.tile_pool(name="epool", bufs=8) as epool, \
             tc.tile_pool(name="norm", bufs=4) as norm, \
             tc.tile_pool(name="psP", bufs=2, space="PSUM") as psP, \
             tc.tile_pool(name="psS", bufs=2, space="PSUM") as psS, \
             tc.tile_pool(name="psY", bufs=1, space="PSUM") as psY:
            tri = consts.tile([128, 128], BF16)
            nc.sync.dma_start(tri[:], tri_dram.ap())

            # input tiles (persistent): x.T and the three weights
            xT = [xw.tile([128, S], BF16, tag=f"xT{i}", name=f"xT{i}")
                  for i in range(N_CT)]
            WK = [xw.tile([128, D_LOC], BF16, tag=f"WK{i}", name=f"WK{i}")
                  for i in range(N_CT)]
            WQ = [xw.tile([128, D_LOC], BF16, tag=f"WQ{i}", name=f"WQ{i}")
                  for i in range(N_CT)]
            WV = [xw.tile([128, D_LOC], BF16, tag=f"WV{i}", name=f"WV{i}")
                  for i in range(N_CT)]
            # K first (needed first), interleaved with x
            for ct in range(N_CT):
                nc.sync.dma_start(xT[ct][:], xtd[ct * 128:(ct + 1) * 128, :])
                nc.sync.dma_start(WK[ct][:], wkt[ct * 128:(ct + 1) * 128, :])
            for ct in range(N_CT):
                nc.sync.dma_start(WQ[ct][:], wqt[ct * 128:(ct + 1) * 128, :])
            for ct in range(N_CT):
                nc.sync.dma_start(WV[ct][:], wvt[ct * 128:(ct + 1) * 128, :])

            # persistent per-core tensors
            QT = [pers.tile([128, S], BF16, tag=f"QT{i}", name=f"QT{i}")
                  for i in range(4)]
            KT = [pers.tile([128, S], BF16, tag=f"KT{i}", name=f"KT{i}")
                  for i in range(4)]
            VP = [pers.tile([128, H_LOC, DH + 1], BF16, tag=f"VP{i}",
                            name=f"VP{i}") for i in range(N_ST)]

            def proj_qk(dst, W, g, qc):
                # dst[g][:, qc*512:(qc+1)*512] = W[:, g-block].T @ x.T chunk
                pp = psP.tile([128, 512], F32, tag="pp", name="pp")
                for ct in range(N_CT):
                    nc.tensor.matmul(
                        pp[:],
                        W[ct][:, g * 128:(g + 1) * 128],
                        xT[ct][:, qc * 512:(qc + 1) * 512],
                        start=(ct == 0), stop=(ct == N_CT - 1))
                nc.vector.tensor_copy(
                    dst[g][:, qc * 512:(qc + 1) * 512], pp[:])

            def proj_v(st):
                pp = psP.tile([128, 512], F32, tag="pp", name="pp")
                for ct in range(N_CT):
                    nc.tensor.matmul(
                        pp[:],
                        xT[ct][:, st * 128:(st + 1) * 128],
                        WV[ct][:],
                        start=(ct == 0), stop=(ct == N_CT - 1))
                nc.vector.tensor_copy(
                    VP[st][:, :, 0:DH],
                    pp[:].rearrange("p (h d) -> p h d", h=H_LOC))
                nc.vector.memset(VP[st][:, :, DH:DH + 1], 1.0)

            def next_step(g, qt):
                if qt < N_QT - 1:
                    return (g, qt + 1)
                if g < 3:
                    return (g + 1, 0)
                return None

            # pipeline fill: first attention step's K/Q chunks
            proj_qk(KT, WK, 0, 0)
            proj_qk(QT, WQ, 0, 0)

            for g in range(4):          # head pair: local heads 2g, 2g+1
                for qt in range(N_QT):  # q-chunk of 512
                    n_kt = 4 * (qt + 1)
                    q0 = qt * 512
                    last = (g == 3 and qt == N_QT - 1)
                    yy = [psY.tile([DH + 1, 512], F32, tag=f"y{hh}",
                                   name=f"y{hh}") for hh in range(2)]

                    def norm_emit(hh, c0, c1):
                        w = c1 - c0
                        den = norm.tile([1, w], F32, tag="den", name="den")
                        nc.vector.tensor_copy(den[:], yy[hh][DH:DH + 1, c0:c1])
                        rd = norm.tile([1, w], F32, tag="rd", name="rd")
                        nc.vector.reciprocal_approx_fast(rd[:], den[:])
                        rdb = norm.tile([DH, w], F32, tag="rdb", name="rdb")
                        nc.gpsimd.partition_broadcast(rdb[:], rd[:])
                        yn = norm.tile([DH, w], BF16, tag="yn", name="yn")
                        nc.vector.tensor_mul(yn[:], yy[hh][0:DH, c0:c1],
                                             rdb[:])
                        nc.sync.dma_start(
                            out[2 * g + hh, :, q0 + c0:q0 + c1], yn[:])

                    for kt in range(n_kt):
                        off = max(0, (kt - 4 * qt) * 128)
                        diag = kt >= 4 * qt
                        # one PSUM tile per k-tile: cols 0:512 head A,
                        # 512:1024 head B
                        ps2 = psS.tile([128, 1024], F32, tag="s", name="s")
                        for hh in range(2):
                            rows = slice(hh * 64, hh * 64 + 64)
                            nc.tensor.matmul(
                                ps2[:, hh * 512 + off:(hh + 1) * 512],
                                KT[g][rows, kt * 128:(kt + 1) * 128],
                                QT[g][rows, q0 + off:q0 + 512],
                                start=True, stop=True)
                        if kt == 0:
                            # PE work that overlaps the exp latency of the
                            # scores just issued: V projections (g==0 only)
                            # and the next attention step's K/Q chunks.
                            if g == 0:
                                for st in range(4 * qt, 4 * qt + 2):
                                    proj_v(st)
                            nxt = next_step(g, qt)
                            if nxt is not None:
                                proj_qk(KT, WK, nxt[0], nxt[1])
                        if kt == 1:
                            if g == 0:
                                for st in range(4 * qt + 2, 4 * qt + 4):
                                    proj_v(st)
                            nxt = next_step(g, qt)
                            if nxt is not None:
                                proj_qk(QT, WQ, nxt[0], nxt[1])
                        ee = epool.tile([128, 1024], BF16, tag="e", name="e")
                        if not diag:
                            nc.scalar.activation(ee[:], ps2[:], EXP,
                                                 scale=SCALE)
                        else:
                            for hh in range(2):
                                cb = hh * 512
                                nc.scalar.activation(
                                    ee[:, cb + off:cb + 512],
                                    ps2[:, cb + off:cb + 512],
                                    EXP, scale=SCALE)
                                nc.vector.tensor_mul(
                                    ee[:, cb + off:cb + off + 128],
                                    ee[:, cb + off:cb + off + 128],
                                    tri[:])
                        for hh in range(2):
                            nc.tensor.matmul(
                                yy[hh][:, off:512],
                                VP[kt][:, 2 * g + hh, :],
                                ee[:, hh * 512 + off:(hh + 1) * 512],
                                start=(kt == 0), stop=(kt == n_kt - 1),
                                skip_group_check=True)
                    if not last:
                        for hh in range(2):
                            norm_emit(hh, 0, 512)
                    else:
                        # final step: de-chained norm in 256-col blocks —
                        # all copies+recips first, then the broadcasts,
                        # then the multiplies, so the in-order DVE stream
                        # never stalls waiting on a GpSimd broadcast
                        chunks = [(hh, c0, c0 + 256)
                                  for c0 in (0, 256) for hh in range(2)]
                        rds, rdbs = [], []
                        for hh, c0, c1 in chunks:
                            den = norm.tile([1, 256], F32, tag="den",
                                            name="den")
                            nc.vector.tensor_copy(
                                den[:], yy[hh][DH:DH + 1, c0:c1])
                            rd = norm.tile([1, 256], F32, tag="rd",
                                           name="rd", bufs=4)
                            nc.vector.reciprocal_approx_fast(rd[:], den[:])
                            rds.append(rd)
                        for i, (hh, c0, c1) in enumerate(chunks):
                            rdb = norm.tile([DH, 256], F32, tag="rdb",
                                            name="rdb", bufs=4)
                            nc.gpsimd.partition_broadcast(rdb[:], rds[i][:])
                            rdbs.append(rdb)
                        for i, (hh, c0, c1) in enumerate(chunks):
                            yn = norm.tile([DH, 256], BF16, tag="yn",
                                           name="yn")
                            nc.vector.tensor_mul(
                                yn[:], yy[hh][0:DH, c0:c1], rdbs[i][:])
                            nc.sync.dma_start(
                                out[2 * g + hh, :, q0 + c0:q0 + c1], yn[:])
    nc.compile()
    return nc


def get_nc():
    if "nc" not in _NC_CACHE:
        _NC_CACHE["nc"] = build_nc()
    return _NC_CACHE["nc"]


def make_in_maps(x, W_q, W_k, W_v):
    in_maps = []
    for core in range(N_CORES):
        b, hg = core // 2, core % 2
        rows = slice(hg * D_LOC, (hg + 1) * D_LOC)
        bf = ml_dtypes.bfloat16
        in_maps.append({
            "xtd": np.ascontiguousarray(np.asarray(x[b], dtype=np.float32).T.astype(bf)),
            "wqt": np.ascontiguousarray(np.asarray(W_q[rows], dtype=np.float32).T.astype(bf)),
            "wkt": np.ascontiguousarray(np.asarray(W_k[rows], dtype=np.float32).T.astype(bf)),
            "wvt": np.ascontiguousarray(np.asarray(W_v[rows], dtype=np.float32).T.astype(bf)),
        })
    return in_maps


def assemble(results):
    Y = np.empty((B, H, S, DH), dtype=np.float32)
    for core in range(N_CORES):
        b, hg = core // 2, core % 2
        yc = np.asarray(results[core]["out"], dtype=np.float32)  # [H_LOC, DH, S]
        Y[b, hg * H_LOC:(hg + 1) * H_LOC] = yc.transpose(0, 2, 1)
    return Y


def kernel(x, W_q, W_k, W_v):
    nc = get_nc()
    in_maps = make_in_maps(x, W_q, W_k, W_v)
    res = run_bass_kernel_spmd(nc, in_maps, list(range(N_CORES)))
    return assemble(res.results)


# revision 19
# speedup vs baseline: 1.0598x; 1.0329x over previous
"""Multihead causal attention on 8 TRN2 NeuronCores.

Sharding: core = (batch b, head-group hg): b = core//2, hg = core%2.
Each core gets x[b] (full sequence, [2048, 1024]) plus the weight rows for
its 8 heads (W[hg*512:(hg+1)*512, :]), computes Q/K/V projections and
causal attention for those (batch, head) pairs, and writes Y transposed
as [8, 64, 2048] (head, dh, seq) in bf16; the host transposes back and
upcasts on gather.

On-device dataflow (per core):
  - x.T supplied pre-transposed by the host (bf16), weights pre-transposed
    likewise; all matmuls bf16.
  - Projections interleaved with attention per head-pair g: K/Q chunks for
    (g, q-chunk qt) are projected just before attention consumes them, so
    the Scalar engine's exp stream starts ~4us into the kernel instead of
    after all projections.
  - Scores in transposed layout scoresT[k, q] = K @ Q.T per head, two heads
    per 128-row group (K=64 each). One [128, 1024] PSUM tile per k-tile
    holds BOTH heads' scores (cols 0:512 head A, 512:1024 head B), so the
    psS pool's two buffers give one full k-tile of score/exp slack and the
    PE never stalls on the exp of the previous k-tile.
  - Causal trimming at 128-column granularity: diagonal k-tiles only
    compute/exp/stream q >= k, so no zero-prefix memsets are needed (PV
    streams are trimmed identically and never read the unwritten prefix).
  - Softmax without a max pass; exp on ScalarE (PSUM -> bf16 SBUF), one
    instruction per off-diagonal k-tile covering both heads; diagonal
    128x128 blocks masked with a triangular 0/1 multiply on DVE.
  - PV matmul in bf16 with a ones-column appended to V: out [65, 512]
    rows 0..63 = unnormalized Y.T, row 64 = softmax denominator.
  - Normalize: denominator copy + fast reciprocal on DVE (reciprocal must
    not read PSUM directly on hardware), partition-broadcast on GpSimd,
    one DVE multiply (fp32 -> bf16), DMA out. The final step is normalized
    in four column chunks to keep the tail off the single-queue DMA rate.
"""
import numpy as np
import ml_dtypes

import concourse.bass as bass
import concourse.tile as tile
from concourse import bacc, mybir
from concourse.bass_utils import run_bass_kernel_spmd

F32 = mybir.dt.float32
BF16 = mybir.dt.bfloat16
EXP = mybir.ActivationFunctionType.Exp

B, S, D, H, DH = 4, 2048, 1024, 16, 64
N_CORES = 8
H_LOC = 8          # heads per core
D_LOC = H_LOC * DH  # 512: projection output dim per core
N_CT = D // 128     # 8 contraction tiles
N_ST = S // 128     # 16 sequence tiles of 128
N_QT = S // 512     # 4 q-chunks of 512
SCALE = 1.0 / np.sqrt(DH)

_NC_CACHE = {}


def build_nc():
    nc = bacc.Bacc("TRN2", target_bir_lowering=False, debug=False,
                   num_devices=N_CORES)
    xtd = nc.dram_tensor("xtd", [D, S], BF16, kind="ExternalInput").ap()
    wqt = nc.dram_tensor("wqt", [D, D_LOC], BF16, kind="ExternalInput").ap()
    wkt = nc.dram_tensor("wkt", [D, D_LOC], BF16, kind="ExternalInput").ap()
    wvt = nc.dram_tensor("wvt", [D, D_LOC], BF16, kind="ExternalInput").ap()
    out = nc.dram_tensor("out", [H_LOC, DH, S], BF16,
                         kind="ExternalOutput").ap()

    # tri[kk, qq] = 1 iff qq >= kk (valid: query position >= key position),
    # duplicated side by side so one DVE multiply masks both heads' planes
    tri_np = (np.arange(128)[None, :] >= np.arange(128)[:, None])
    tri2_np = np.concatenate([tri_np, tri_np], axis=1)
    tri_dram = nc.inline_tensor(tri2_np.astype(ml_dtypes.bfloat16),
                                name="tri")

    with tile.TileContext(nc) as tc:
        with tc.tile_pool(name="consts", bufs=1) as consts, \
             tc.tile_pool(name="pers", bufs=1) as pers, \
             tc.tile_pool(name="xw", bufs=1) as xw, \
             tc.tile_pool(name="epool", bufs=8) as epool, \
             tc.tile_pool(name="norm", bufs=4) as norm, \
             tc.tile_pool(name="psP", bufs=2, space="PSUM") as psP, \
             tc.tile_pool(name="psS", bufs=2, space="PSUM") as psS, \
             tc.tile_pool(name="psY", bufs=1, space="PSUM") as psY:
            tri = consts.tile([128, 2, 128], BF16)
            nc.sync.dma_start(tri[:], tri_dram.ap())

            # input tiles (persistent): x.T and the three weights
            xT = [xw.tile([128, S], BF16, tag=f"xT{i}", name=f"xT{i}")
                  for i in range(N_CT)]
            WK = [xw.tile([128, D_LOC], BF16, tag=f"WK{i}", name=f"WK{i}")
                  for i in range(N_CT)]
            WQ = [xw.tile([128, D_LOC], BF16, tag=f"WQ{i}", name=f"WQ{i}")
                  for i in range(N_CT)]
            WV = [xw.tile([128, D_LOC], BF16, tag=f"WV{i}", name=f"WV{i}")
                  for i in range(N_CT)]
            # K first (needed first), interleaved with x
            for ct in range(N_CT):
                nc.sync.dma_start(xT[ct][:], xtd[ct * 128:(ct + 1) * 128, :])
                nc.sync.dma_start(WK[ct][:], wkt[ct * 128:(ct + 1) * 128, :])
            for ct in range(N_CT):
                nc.sync.dma_start(WQ[ct][:], wqt[ct * 128:(ct + 1) * 128, :])
            for ct in range(N_CT):
                nc.sync.dma_start(WV[ct][:], wvt[ct * 128:(ct + 1) * 128, :])

            # persistent per-core tensors
            QT = [pers.tile([128, S], BF16, tag=f"QT{i}", name=f"QT{i}")
                  for i in range(4)]
            KT = [pers.tile([128, S], BF16, tag=f"KT{i}", name=f"KT{i}")
                  for i in range(4)]
            VP = [pers.tile([128, H_LOC, DH + 1], BF16, tag=f"VP{i}",
                            name=f"VP{i}") for i in range(N_ST)]

            def proj_qk(dst, W, g, qc):
                # dst[g][:, qc*512:(qc+1)*512] = W[:, g-block].T @ x.T chunk
                pp = psP.tile([128, 512], F32, tag="pp", name="pp")
                for ct in range(N_CT):
                    nc.tensor.matmul(
                        pp[:],
                        W[ct][:, g * 128:(g + 1) * 128],
                        xT[ct][:, qc * 512:(qc + 1) * 512],
                        start=(ct == 0), stop=(ct == N_CT - 1))
                nc.vector.tensor_copy(
                    dst[g][:, qc * 512:(qc + 1) * 512], pp[:])

            def proj_v(st):
                pp = psP.tile([128, 512], F32, tag="pp", name="pp")
                for ct in range(N_CT):
                    nc.tensor.matmul(
                        pp[:],
                        xT[ct][:, st * 128:(st + 1) * 128],
                        WV[ct][:],
                        start=(ct == 0), stop=(ct == N_CT - 1))
                nc.vector.tensor_copy(
                    VP[st][:, :, 0:DH],
                    pp[:].rearrange("p (h d) -> p h d", h=H_LOC))
                nc.vector.memset(VP[st][:, :, DH:DH + 1], 1.0)

            def next_step(g, qt):
                if qt < N_QT - 1:
                    return (g, qt + 1)
                if g < 3:
                    return (g + 1, 0)
                return None

            # pipeline fill: first attention step's K/Q chunks
            proj_qk(KT, WK, 0, 0)
            proj_qk(QT, WQ, 0, 0)

            for g in range(4):          # head pair: local heads 2g, 2g+1
                for qt in range(N_QT):  # q-chunk of 512
                    n_kt = 4 * (qt + 1)
                    q0 = qt * 512
                    last = (g == 3 and qt == N_QT - 1)
                    yy = [psY.tile([DH + 1, 512], F32, tag=f"y{hh}",
                                   name=f"y{hh}") for hh in range(2)]

                    def norm_emit(hh, c0, c1):
                        w = c1 - c0
                        den = norm.tile([1, w], F32, tag="den", name="den")
                        nc.vector.tensor_copy(den[:], yy[hh][DH:DH + 1, c0:c1])
                        rd = norm.tile([1, w], F32, tag="rd", name="rd")
                        nc.vector.reciprocal_approx_fast(rd[:], den[:])
                        rdb = norm.tile([DH, w], F32, tag="rdb", name="rdb")
                        nc.gpsimd.partition_broadcast(rdb[:], rd[:])
                        yn = norm.tile([DH, w], BF16, tag="yn", name="yn")
                        nc.vector.tensor_mul(yn[:], yy[hh][0:DH, c0:c1],
                                             rdb[:])
                        nc.sync.dma_start(
                            out[2 * g + hh, :, q0 + c0:q0 + c1], yn[:])

                    for kt in range(n_kt):
                        off = max(0, (kt - 4 * qt) * 128)
                        diag = kt >= 4 * qt
                        # one PSUM tile per k-tile: plane 0 head A,
                        # plane 1 head B
                        ps2 = psS.tile([128, 2, 512], F32, tag="s", name="s")
                        for hh in range(2):
                            rows = slice(hh * 64, hh * 64 + 64)
                            nc.tensor.matmul(
                                ps2[:, hh, off:512],
                                KT[g][rows, kt * 128:(kt + 1) * 128],
                                QT[g][rows, q0 + off:q0 + 512],
                                start=True, stop=True)
                        if kt == 0:
                            # PE work that overlaps the exp latency of the
                            # scores just issued: V projections (g==0 only)
                            # and the next attention step's K/Q chunks.
                            if g == 0:
                                for st in range(4 * qt, 4 * qt + 2):
                                    proj_v(st)
                            nxt = next_step(g, qt)
                            if nxt is not None:
                                proj_qk(KT, WK, nxt[0], nxt[1])
                        if kt == 1:
                            if g == 0:
                                for st in range(4 * qt + 2, 4 * qt + 4):
                                    proj_v(st)
                            nxt = next_step(g, qt)
                            if nxt is not None:
                                proj_qk(QT, WQ, nxt[0], nxt[1])
                        ee = epool.tile([128, 2, 512], BF16, tag="e",
                                        name="e")
                        # one exp instruction covers both heads' planes
                        # (3D strided AP), trimmed to the causal region
                        nc.scalar.activation(ee[:, :, off:512],
                                             ps2[:, :, off:512], EXP,
                                             scale=SCALE)
                        if diag:
                            nc.vector.tensor_mul(
                                ee[:, :, off:off + 128],
                                ee[:, :, off:off + 128],
                                tri[:])
                        for hh in range(2):
                            nc.tensor.matmul(
                                yy[hh][:, off:512],
                                VP[kt][:, 2 * g + hh, :],
                                ee[:, hh, off:512],
                                start=(kt == 0), stop=(kt == n_kt - 1),
                                skip_group_check=True)
                    if not last:
                        for hh in range(2):
                            norm_emit(hh, 0, 512)
                    else:
                        # final step: de-chained norm in 256-col blocks —
                        # all copies+recips first, then the broadcasts,
                        # then the multiplies, so the in-order DVE stream
                        # never stalls waiting on a GpSimd broadcast
                        chunks = [(hh, c0, c0 + 256)
                                  for c0 in (0, 256) for hh in range(2)]
                        rds, rdbs = [], []
                        for hh, c0, c1 in chunks:
                            den = norm.tile([1, 256], F32, tag="den",
                                            name="den")
                            nc.vector.tensor_copy(
                                den[:], yy[hh][DH:DH + 1, c0:c1])
                            rd = norm.tile([1, 256], F32, tag="rd",
                                           name="rd", bufs=4)
                            nc.vector.reciprocal_approx_fast(rd[:], den[:])
                            rds.append(rd)
                        for i, (hh, c0, c1) in enumerate(chunks):
                            rdb = norm.tile([DH, 256], F32, tag="rdb",
                                            name="rdb", bufs=4)
                            nc.gpsimd.partition_broadcast(rdb[:], rds[i][:])
                            rdbs.append(rdb)
                        for i, (hh, c0, c1) in enumerate(chunks):
                            yn = norm.tile([DH, 256], BF16, tag="yn",
                                           name="yn")
                            nc.vector.tensor_mul(
                                yn[:], yy[hh][0:DH, c0:c1], rdbs[i][:])
                            nc.sync.dma_start(
                                out[2 * g + hh, :, q0 + c0:q0 + c1], yn[:])
    nc.compile()
    return nc


def get_nc():
    if "nc" not in _NC_CACHE:
        _NC_CACHE["nc"] = build_nc()
    return _NC_CACHE["nc"]


def make_in_maps(x, W_q, W_k, W_v):
    in_maps = []
    for core in range(N_CORES):
        b, hg = core // 2, core % 2
        rows = slice(hg * D_LOC, (hg + 1) * D_LOC)
        bf = ml_dtypes.bfloat16
        in_maps.append({
            "xtd": np.ascontiguousarray(np.asarray(x[b], dtype=np.float32).T.astype(bf)),
            "wqt": np.ascontiguousarray(np.asarray(W_q[rows], dtype=np.float32).T.astype(bf)),
            "wkt": np.ascontiguousarray(np.asarray(W_k[rows], dtype=np.float32).T.astype(bf)),
            "wvt": np.ascontiguousarray(np.asarray(W_v[rows], dtype=np.float32).T.astype(bf)),
        })
    return in_maps


def assemble(results):
    Y = np.empty((B, H, S, DH), dtype=np.float32)
    for core in range(N_CORES):
        b, hg = core // 2, core % 2
        yc = np.asarray(results[core]["out"], dtype=np.float32)  # [H_LOC, DH, S]
        Y[b, hg * H_LOC:(hg + 1) * H_LOC] = yc.transpose(0, 2, 1)
    return Y


def kernel(x, W_q, W_k, W_v):
    nc = get_nc()
    in_maps = make_in_maps(x, W_q, W_k, W_v)
    res = run_bass_kernel_spmd(nc, in_maps, list(range(N_CORES)))
    return assemble(res.results)


# revision 20
# speedup vs baseline: 1.0701x; 1.0097x over previous
"""Multihead causal attention on 8 TRN2 NeuronCores.

Sharding: core = (batch b, head-group hg): b = core//2, hg = core%2.
Each core gets x[b] (full sequence, [2048, 1024]) plus the weight rows for
its 8 heads (W[hg*512:(hg+1)*512, :]), computes Q/K/V projections and
causal attention for those (batch, head) pairs, and writes Y transposed
as [8, 64, 2048] (head, dh, seq) in bf16; the host transposes back and
upcasts on gather.

On-device dataflow (per core):
  - x.T supplied pre-transposed by the host (bf16), weights pre-transposed
    likewise; all matmuls bf16.
  - Projections interleaved with attention per head-pair g: K/Q chunks for
    (g, q-chunk qt) are projected just before attention consumes them, so
    the Scalar engine's exp stream starts ~4us into the kernel instead of
    after all projections.
  - Scores in transposed layout scoresT[k, q] = K @ Q.T per head, two heads
    per 128-row group (K=64 each). One [128, 1024] PSUM tile per k-tile
    holds BOTH heads' scores (cols 0:512 head A, 512:1024 head B), so the
    psS pool's two buffers give one full k-tile of score/exp slack and the
    PE never stalls on the exp of the previous k-tile.
  - Causal trimming at 128-column granularity: diagonal k-tiles only
    compute/exp/stream q >= k, so no zero-prefix memsets are needed (PV
    streams are trimmed identically and never read the unwritten prefix).
  - Softmax without a max pass; exp on ScalarE (PSUM -> bf16 SBUF), one
    instruction per off-diagonal k-tile covering both heads; diagonal
    128x128 blocks masked with a triangular 0/1 multiply on DVE.
  - PV matmul in bf16 with a ones-column appended to V: out [65, 512]
    rows 0..63 = unnormalized Y.T, row 64 = softmax denominator.
  - Normalize: denominator copy + fast reciprocal on DVE (reciprocal must
    not read PSUM directly on hardware), partition-broadcast on GpSimd,
    one DVE multiply (fp32 -> bf16), DMA out. The final step is normalized
    in four column chunks to keep the tail off the single-queue DMA rate.
"""
import numpy as np
import ml_dtypes

import concourse.bass as bass
import concourse.tile as tile
from concourse import bacc, mybir
from concourse.bass_utils import run_bass_kernel_spmd

F32 = mybir.dt.float32
BF16 = mybir.dt.bfloat16
EXP = mybir.ActivationFunctionType.Exp

B, S, D, H, DH = 4, 2048, 1024, 16, 64
N_CORES = 8
H_LOC = 8          # heads per core
D_LOC = H_LOC * DH  # 512: projection output dim per core
N_CT = D // 128     # 8 contraction tiles
N_ST = S // 128     # 16 sequence tiles of 128
N_QT = S // 512     # 4 q-chunks of 512
SCALE = 1.0 / np.sqrt(DH)

_NC_CACHE = {}


def build_nc():
    nc = bacc.Bacc("TRN2", target_bir_lowering=False, debug=False,
                   num_devices=N_CORES)
    xtd = nc.dram_tensor("xtd", [D, S], BF16, kind="ExternalInput").ap()
    wqt = nc.dram_tensor("wqt", [D, D_LOC], BF16, kind="ExternalInput").ap()
    wkt = nc.dram_tensor("wkt", [D, D_LOC], BF16, kind="ExternalInput").ap()
    wvt = nc.dram_tensor("wvt", [D, D_LOC], BF16, kind="ExternalInput").ap()
    out = nc.dram_tensor("out", [H_LOC, DH, S], BF16,
                         kind="ExternalOutput").ap()

    # tri[kk, qq] = 1 iff qq >= kk (valid: query position >= key position),
    # duplicated side by side so one DVE multiply masks both heads' planes
    tri_np = (np.arange(128)[None, :] >= np.arange(128)[:, None])
    tri2_np = np.concatenate([tri_np, tri_np], axis=1)
    tri_dram = nc.inline_tensor(tri2_np.astype(ml_dtypes.bfloat16),
                                name="tri")

    with tile.TileContext(nc) as tc:
        with tc.tile_pool(name="consts", bufs=1) as consts, \
             tc.tile_pool(name="pers", bufs=1) as pers, \
             tc.tile_pool(name="xw", bufs=1) as xw, \
             tc.tile_pool(name="epool", bufs=8) as epool, \
             tc.tile_pool(name="norm", bufs=4) as norm, \
             tc.tile_pool(name="psP", bufs=2, space="PSUM") as psP, \
             tc.tile_pool(name="psS", bufs=2, space="PSUM") as psS, \
             tc.tile_pool(name="psY", bufs=1, space="PSUM") as psY:
            tri = consts.tile([128, 2, 128], BF16)
            nc.sync.dma_start(tri[:], tri_dram.ap())

            # input tiles (persistent): x.T and the three weights
            xT = [xw.tile([128, S], BF16, tag=f"xT{i}", name=f"xT{i}")
                  for i in range(N_CT)]
            WK = [xw.tile([128, D_LOC], BF16, tag=f"WK{i}", name=f"WK{i}")
                  for i in range(N_CT)]
            WQ = [xw.tile([128, D_LOC], BF16, tag=f"WQ{i}", name=f"WQ{i}")
                  for i in range(N_CT)]
            WV = [xw.tile([128, D_LOC], BF16, tag=f"WV{i}", name=f"WV{i}")
                  for i in range(N_CT)]
            # x is consumed column-chunked (q-chunks of 512): DMA it in
            # [128, 512] pieces ordered by first use, so the first
            # projection chunk only waits for ~2MB, not the whole input
            def dma_x(ct, qc):
                nc.sync.dma_start(
                    xT[ct][:, qc * 512:(qc + 1) * 512],
                    xtd[ct * 128:(ct + 1) * 128, qc * 512:(qc + 1) * 512])
            for ct in range(N_CT):
                dma_x(ct, 0)
                nc.sync.dma_start(WK[ct][:], wkt[ct * 128:(ct + 1) * 128, :])
            for ct in range(N_CT):
                nc.sync.dma_start(WQ[ct][:], wqt[ct * 128:(ct + 1) * 128, :])
            for ct in range(N_CT):
                nc.sync.dma_start(WV[ct][:], wvt[ct * 128:(ct + 1) * 128, :])
                dma_x(ct, 1)
            for ct in range(N_CT):
                dma_x(ct, 2)
                dma_x(ct, 3)

            # persistent per-core tensors
            QT = [pers.tile([128, S], BF16, tag=f"QT{i}", name=f"QT{i}")
                  for i in range(4)]
            KT = [pers.tile([128, S], BF16, tag=f"KT{i}", name=f"KT{i}")
                  for i in range(4)]
            VP = [pers.tile([128, H_LOC, DH + 1], BF16, tag=f"VP{i}",
                            name=f"VP{i}") for i in range(N_ST)]

            def proj_qk(dst, W, g, qc):
                # dst[g][:, qc*512:(qc+1)*512] = W[:, g-block].T @ x.T chunk
                pp = psP.tile([128, 512], F32, tag="pp", name="pp")
                for ct in range(N_CT):
                    nc.tensor.matmul(
                        pp[:],
                        W[ct][:, g * 128:(g + 1) * 128],
                        xT[ct][:, qc * 512:(qc + 1) * 512],
                        start=(ct == 0), stop=(ct == N_CT - 1))
                nc.vector.tensor_copy(
                    dst[g][:, qc * 512:(qc + 1) * 512], pp[:])

            def proj_v(st):
                pp = psP.tile([128, 512], F32, tag="pp", name="pp")
                for ct in range(N_CT):
                    nc.tensor.matmul(
                        pp[:],
                        xT[ct][:, st * 128:(st + 1) * 128],
                        WV[ct][:],
                        start=(ct == 0), stop=(ct == N_CT - 1))
                nc.vector.tensor_copy(
                    VP[st][:, :, 0:DH],
                    pp[:].rearrange("p (h d) -> p h d", h=H_LOC))
                nc.vector.memset(VP[st][:, :, DH:DH + 1], 1.0)

            def next_step(g, qt):
                if qt < N_QT - 1:
                    return (g, qt + 1)
                if g < 3:
                    return (g + 1, 0)
                return None

            # pipeline fill: first attention step's K/Q chunks
            proj_qk(KT, WK, 0, 0)
            proj_qk(QT, WQ, 0, 0)

            for g in range(4):          # head pair: local heads 2g, 2g+1
                for qt in range(N_QT):  # q-chunk of 512
                    n_kt = 4 * (qt + 1)
                    q0 = qt * 512
                    last = (g == 3 and qt == N_QT - 1)
                    yy = [psY.tile([DH + 1, 512], F32, tag=f"y{hh}",
                                   name=f"y{hh}") for hh in range(2)]

                    def norm_emit(hh, c0, c1):
                        w = c1 - c0
                        den = norm.tile([1, w], F32, tag="den", name="den")
                        nc.vector.tensor_copy(den[:], yy[hh][DH:DH + 1, c0:c1])
                        rd = norm.tile([1, w], F32, tag="rd", name="rd")
                        nc.vector.reciprocal_approx_fast(rd[:], den[:])
                        rdb = norm.tile([DH, w], F32, tag="rdb", name="rdb")
                        nc.gpsimd.partition_broadcast(rdb[:], rd[:])
                        yn = norm.tile([DH, w], BF16, tag="yn", name="yn")
                        nc.vector.tensor_mul(yn[:], yy[hh][0:DH, c0:c1],
                                             rdb[:])
                        nc.sync.dma_start(
                            out[2 * g + hh, :, q0 + c0:q0 + c1], yn[:])

                    for kt in range(n_kt):
                        off = max(0, (kt - 4 * qt) * 128)
                        diag = kt >= 4 * qt
                        # one PSUM tile per k-tile: plane 0 head A,
                        # plane 1 head B
                        ps2 = psS.tile([128, 2, 512], F32, tag="s", name="s")
                        for hh in range(2):
                            rows = slice(hh * 64, hh * 64 + 64)
                            nc.tensor.matmul(
                                ps2[:, hh, off:512],
                                KT[g][rows, kt * 128:(kt + 1) * 128],
                                QT[g][rows, q0 + off:q0 + 512],
                                start=True, stop=True)
                        if kt == 0:
                            # PE work that overlaps the exp latency of the
                            # scores just issued: V projections (g==0 only)
                            # and the next attention step's K/Q chunks.
                            if g == 0:
                                for st in range(4 * qt, 4 * qt + 2):
                                    proj_v(st)
                            nxt = next_step(g, qt)
                            if nxt is not None:
                                proj_qk(KT, WK, nxt[0], nxt[1])
                        if kt == 1:
                            if g == 0:
                                for st in range(4 * qt + 2, 4 * qt + 4):
                                    proj_v(st)
                            nxt = next_step(g, qt)
                            if nxt is not None:
                                proj_qk(QT, WQ, nxt[0], nxt[1])
                        ee = epool.tile([128, 2, 512], BF16, tag="e",
                                        name="e")
                        # one exp instruction covers both heads' planes
                        # (3D strided AP), trimmed to the causal region
                        nc.scalar.activation(ee[:, :, off:512],
                                             ps2[:, :, off:512], EXP,
                                             scale=SCALE)
                        if diag:
                            nc.vector.tensor_mul(
                                ee[:, :, off:off + 128],
                                ee[:, :, off:off + 128],
                                tri[:])
                        for hh in range(2):
                            nc.tensor.matmul(
                                yy[hh][:, off:512],
                                VP[kt][:, 2 * g + hh, :],
                                ee[:, hh, off:512],
                                start=(kt == 0), stop=(kt == n_kt - 1),
                                skip_group_check=True)
                    if not last:
                        for hh in range(2):
                            norm_emit(hh, 0, 512)
                    else:
                        # final step: de-chained norm in 256-col blocks —
                        # all copies+recips first, then the broadcasts,
                        # then the multiplies, so the in-order DVE stream
                        # never stalls waiting on a GpSimd broadcast
                        chunks = [(hh, c0, c0 + 256)
                                  for c0 in (0, 256) for hh in range(2)]
                        rds, rdbs = [], []
                        for hh, c0, c1 in chunks:
                            den = norm.tile([1, 256], F32, tag="den",
                                            name="den")
                            nc.vector.tensor_copy(
                                den[:], yy[hh][DH:DH + 1, c0:c1])
                            rd = norm.tile([1, 256], F32, tag="rd",
                                           name="rd", bufs=4)
                            nc.vector.reciprocal_approx_fast(rd[:], den[:])
                            rds.append(rd)
                        for i, (hh, c0, c1) in enumerate(chunks):
                            rdb = norm.tile([DH, 256], F32, tag="rdb",
                                            name="rdb", bufs=4)
                            nc.gpsimd.partition_broadcast(rdb[:], rds[i][:])
                            rdbs.append(rdb)
                        for i, (hh, c0, c1) in enumerate(chunks):
                            yn = norm.tile([DH, 256], BF16, tag="yn",
                                           name="yn")
                            nc.vector.tensor_mul(
                                yn[:], yy[hh][0:DH, c0:c1], rdbs[i][:])
                            nc.sync.dma_start(
                                out[2 * g + hh, :, q0 + c0:q0 + c1], yn[:])
    nc.compile()
    return nc


def get_nc():
    if "nc" not in _NC_CACHE:
        _NC_CACHE["nc"] = build_nc()
    return _NC_CACHE["nc"]


def make_in_maps(x, W_q, W_k, W_v):
    in_maps = []
    for core in range(N_CORES):
        b, hg = core // 2, core % 2
        rows = slice(hg * D_LOC, (hg + 1) * D_LOC)
        bf = ml_dtypes.bfloat16
        in_maps.append({
            "xtd": np.ascontiguousarray(np.asarray(x[b], dtype=np.float32).T.astype(bf)),
            "wqt": np.ascontiguousarray(np.asarray(W_q[rows], dtype=np.float32).T.astype(bf)),
            "wkt": np.ascontiguousarray(np.asarray(W_k[rows], dtype=np.float32).T.astype(bf)),
            "wvt": np.ascontiguousarray(np.asarray(W_v[rows], dtype=np.float32).T.astype(bf)),
        })
    return in_maps


def assemble(results):
    Y = np.empty((B, H, S, DH), dtype=np.float32)
    for core in range(N_CORES):
        b, hg = core // 2, core % 2
        yc = np.asarray(results[core]["out"], dtype=np.float32)  # [H_LOC, DH, S]
        Y[b, hg * H_LOC:(hg + 1) * H_LOC] = yc.transpose(0, 2, 1)
    return Y


def kernel(x, W_q, W_k, W_v):
    nc = get_nc()
    in_maps = make_in_maps(x, W_q, W_k, W_v)
    res = run_bass_kernel_spmd(nc, in_maps, list(range(N_CORES)))
    return assemble(res.results)
